# revision 15
# baseline (speedup 1.0000x reference)
"""DiffFormerBlock Trainium2 kernel.

Data-parallel over batch B=8: core b processes image b end-to-end (no
collectives).  Per-core pipeline, all activations feature-major ("fm",
[C-chunk partitions, tokens]) inside the attention block:

  phase A: LN(x), LN(y) token-major (bn_stats) -> prescale -> PE-transpose
           -> gamma/beta -> X_fm, Y_fm (bf16, window-ordered tokens)
  phase B (layer 1) / phase C (layer 2), per 8-window group, per branch:
           q/k projections (bf16 matmul), v projection (activations
           stationary -> token-major v), scores S^T = k^T q per (head,
           window) with half-masked stationaries (no row tiling), bias add
           on PSUM, exp on ACT, sum-exp via half-ones matmul, reciprocal,
           normalize P^T, P@V via masked v -> O^T feature-major, out-proj
           (fp32r).  Layer 1 writes back into X_fm/Y_fm, layer 2 spills
           fp32 x2/y2 to DRAM scratch.
  phase D: residual + LN2 + MLP (fp32r) + residual, transposes via PE,
           final store in raster order.
"""

import numpy as np

import concourse.bass as bass
import concourse.tile as tile
import concourse.mybir as mybir
from concourse.bass_utils import run_bass_kernel_spmd
from concourse.vector_clock import ScopedClock

F32 = mybir.dt.float32
F32R = mybir.dt.float32r
BF16 = mybir.dt.bfloat16
AF = mybir.ActivationFunctionType
OP = mybir.AluOpType

B = 8
HW = 64            # image height == width
C = 384
CK = 3             # C / 128
NH = 12
HD = 32
WS = 8             # window size
N = 64             # tokens per window
T = 4096           # tokens per image
NWIN = 64          # windows per image
NG = 8             # window groups (one row of 8 windows each)
GW = 8             # windows per group
GT = 512           # tokens per group
MLP_H = 1536
MHK = 12           # MLP_H / 128
SCALE = HD ** -0.5


def _split_excess_waits(nc):
    """The pinned walrus accepts exactly ONE sync-wait command per
    instruction (any opcode).  Hoist surplus waits onto single-wait NoOps
    inserted just before the instruction on the same engine — the engine
    sequencer executes them in order, so the downstream instruction still
    starts only after every semaphore condition is met."""
    ctr = [0]
    for fn in nc.m.functions:
        for blk in fn.blocks:
            out = []
            changed = False
            for inst in blk.instructions:
                si = getattr(inst, "sync_info", None)
                ow = list(si.on_wait) if (si is not None and si.on_wait) else []
                if len(ow) > 1:
                    for w in ow[:-1]:
                        ctr[0] += 1
                        nop = mybir.InstNoOp(
                            name=f"waitnop-{ctr[0]}", ins=[], outs=[])
                        nop.engine = inst.engine
                        nop.sync_info = mybir.SyncInfo(on_wait=[w],
                                                       on_update=[])
                        out.append(nop)
                    inst.sync_info = mybir.SyncInfo(
                        on_wait=[ow[-1]],
                        on_update=list(si.on_update or []))
                    changed = True
                out.append(inst)
            if changed:
                blk.instructions = out


class _PatchedTileContext(tile.TileContext):
    """The pinned walrus accepts at most one sync-wait on a Drain; spread the
    exit drain's extra DMA-queue waits across individual SP nops."""

    def schedule_and_allocate(self):
        res = super().schedule_and_allocate()
        _split_excess_waits(self.nc)
        return res

    def _drain_and_barrier(self, tick_clock, wait_clock):
        drain_inst = self.nc.sync.drain()
        wait_clock.add_sem_waits(
            drain_inst.ins, ScopedClock({None: tick_clock.global_clock})
        )
        si = drain_inst.ins.sync_info
        if si is not None and si.on_wait is not None and len(si.on_wait) > 1:
            waits = list(si.on_wait)
            drain_inst.ins.sync_info = mybir.SyncInfo(
                on_wait=[waits[0]], on_update=list(si.on_update or [])
            )
            for w in waits[1:]:
                nop = self.nc.sync.nop(nofuse=True)
                nop.ins.sync_info = mybir.SyncInfo(on_wait=[w], on_update=[])

        self.nc.all_engine_barrier()
        assert self.sems is not None
        popped = self.nc._tile_sem_poison_stack.pop()
        assert popped is self._sem_poison
        self.nc.clear_and_free_semaphores(list(self.sems.allocated().values()))
        self.nc.all_engine_barrier()


def _win_view(dram_ap):
    """[T, C] raster -> [wh, ih, ww, iw, C]."""
    return dram_ap.rearrange(
        "(wh ih ww iw) c -> wh ih ww iw c", wh=WS, ih=WS, ww=WS, iw=WS
    )


def _win_half(dram_ap, g, tc4, ww):
    """AP for one window (64 tokens) of chunk (g, tc4): [ih, iw, C]."""
    v = _win_view(dram_ap)
    return v[g, :, 2 * tc4 + ww, :, :]


def _dma_win_chunk_in(nc, xr, dram_ap, g, tc4):
    for ww in range(2):
        nc.sync.dma_start(xr[ww * 64:(ww + 1) * 64, :],
                          _win_half(dram_ap, g, tc4, ww))


def _dma_win_chunk_out(nc, dram_ap, ot, g, tc4):
    for ww in range(2):
        nc.sync.dma_start(_win_half(dram_ap, g, tc4, ww),
                          ot[ww * 64:(ww + 1) * 64, :])


BRANCHES = ["a1", "a2", "g1", "g2"]


def build_nc():
    nc = bass.Bass()

    # ---- DRAM I/O ----
    d = {}
    for s in ("x", "y"):
        d[s] = nc.dram_tensor(s, [T, C], F32, kind="ExternalInput")
        d["qg_" + s] = nc.dram_tensor("qg_" + s, [128, CK, N], F32,
                                      kind="ExternalInput")
    for br in BRANCHES:
        d[br + "_qw"] = nc.dram_tensor(br + "_qw", [CK, 128, C], BF16,
                                       kind="ExternalInput")
        d[br + "_kw"] = nc.dram_tensor(br + "_kw", [CK, 128, C], BF16,
                                       kind="ExternalInput")
        d[br + "_vw"] = nc.dram_tensor(br + "_vw", [CK, 128, C], BF16,
                                       kind="ExternalInput")
        d[br + "_pw"] = nc.dram_tensor(br + "_pw", [CK, 128, C], F32R,
                                       kind="ExternalInput")
        d[br + "_qb"] = nc.dram_tensor(br + "_qb", [128, CK], F32,
                                       kind="ExternalInput")
        d[br + "_kb"] = nc.dram_tensor(br + "_kb", [128, CK], F32,
                                       kind="ExternalInput")
        d[br + "_vb"] = nc.dram_tensor(br + "_vb", [1, C], BF16,
                                       kind="ExternalInput")
        d[br + "_pb"] = nc.dram_tensor(br + "_pb", [128, CK], F32,
                                       kind="ExternalInput")
        d[br + "_bt"] = nc.dram_tensor(br + "_bt", [128, NH, N], F32,
                                       kind="ExternalInput")
    for m in ("m1", "m2"):
        d[m + "_w1"] = nc.dram_tensor(m + "_w1", [CK, 128, MLP_H], F32R,
                                      kind="ExternalInput")
        d[m + "_b1"] = nc.dram_tensor(m + "_b1", [128, MHK], F32,
                                      kind="ExternalInput")
        d[m + "_w2"] = nc.dram_tensor(m + "_w2", [MHK, 128, C], F32R,
                                      kind="ExternalInput")
        d[m + "_b2"] = nc.dram_tensor(m + "_b2", [128, CK], F32,
                                      kind="ExternalInput")
    for ln in ("n11", "n12", "n21", "n22"):
        d[ln + "_g"] = nc.dram_tensor(ln + "_g", [128, CK], F32,
                                      kind="ExternalInput")
        d[ln + "_b"] = nc.dram_tensor(ln + "_b", [128, CK], F32,
                                      kind="ExternalInput")
    d["ident_bf"] = nc.dram_tensor("ident_bf", [128, 128], BF16,
                                   kind="ExternalInput")
    d["ident_f32"] = nc.dram_tensor("ident_f32", [128, 128], F32,
                                    kind="ExternalInput")
    d["halfones"] = nc.dram_tensor("halfones", [128, 2, 64], BF16,
                                   kind="ExternalInput")
    d["onesrow"] = nc.dram_tensor("onesrow", [1, 128], BF16,
                                  kind="ExternalInput")

    d["xo"] = nc.dram_tensor("xo", [T, C], F32, kind="ExternalOutput")
    d["yo"] = nc.dram_tensor("yo", [T, C], F32, kind="ExternalOutput")

    # DRAM scratch for layer-2 outputs (feature-major fp32)
    scratch = {
        "x": nc.dram_tensor("x2s", [128, CK, T], F32, kind="Internal"),
        "y": nc.dram_tensor("y2s", [128, CK, T], F32, kind="Internal"),
    }

    with _PatchedTileContext(nc) as tc:
        _emit(nc, tc, d, scratch)
    return nc


def _emit(nc, tc, d, scratch):
    import contextlib

    ctx = contextlib.ExitStack()
    with ctx:
        persist = ctx.enter_context(tc.tile_pool(name="persist", bufs=1))
        # psum pools (<= 8 banks total while attention pools are open)
        ps_t = ctx.enter_context(
            tc.tile_pool(name="ps_t", bufs=2, space="PSUM"))     # transposes
        ps_m = ctx.enter_context(
            tc.tile_pool(name="ps_m", bufs=3, space="PSUM"))     # proj/scores/SE

        # ---- persistent constants ----
        ident_bf = persist.tile([128, 128], BF16)
        nc.sync.dma_start(ident_bf[:], d["ident_bf"][:])
        ident_f = persist.tile([128, 128], F32)
        nc.sync.dma_start(ident_f[:], d["ident_f32"][:])
        halfones = persist.tile([128, 2, 64], BF16)
        nc.sync.dma_start(halfones[:], d["halfones"][:])
        onesrow = persist.tile([1, 128], BF16)
        nc.sync.dma_start(onesrow[:], d["onesrow"][:])
        eps = persist.tile([128, 1], F32)
        nc.vector.memset(eps[:], 1e-5)

        lngb = {}
        for ln in ("n11", "n12", "n21", "n22"):
            for gb in ("_g", "_b"):
                t = persist.tile([128, CK], F32, name=ln + gb)
                nc.sync.dma_start(t[:], d[ln + gb][:])
                lngb[ln + gb] = t
        qg_sb = {}
        for s in ("x", "y"):
            t = persist.tile([128, CK, N], F32, name="qg" + s)
            nc.sync.dma_start(t[:], d["qg_" + s][:])
            qg_sb[s] = t
        bt_sb = {}
        for br in BRANCHES:
            t = persist.tile([128, NH, N], F32, name="bt" + br)
            nc.sync.dma_start(t[:], d[br + "_bt"][:])
            bt_sb[br] = t

        with tc.tile_pool(name="xy", bufs=1) as xyp:
            X = {
                "x": xyp.tile([128, CK, T], BF16, name="X_fm"),
                "y": xyp.tile([128, CK, T], BF16, name="Y_fm"),
            }
            with tc.tile_pool(name="pha", bufs=3) as pha:
                _phase_a(nc, tc, d, lngb, ident_bf, eps, pha, ps_t, X)
            # pool space is reused across phases; hard-fence the transitions
            tc.strict_bb_all_engine_barrier()
            with (
                tc.tile_pool(name="wts", bufs=1) as wts,
                tc.tile_pool(name="attn", bufs=2) as attn,
                tc.tile_pool(name="attn1", bufs=1) as attn1,
                tc.tile_pool(name="ps_pvt", bufs=3, space="PSUM") as ps_pvt,
            ):
                # one-time zero backgrounds for masked stationaries
                kmask = {
                    s: attn1.tile([128, CK, 4, GT], BF16, name="kmask" + s)
                    for s in ("x", "y")
                }
                vpad = {
                    s: attn1.tile([128, GW, NH, HD], BF16, name="vpad" + s)
                    for s in ("x", "y")
                }
                for s in ("x", "y"):
                    nc.vector.memset(kmask[s][:], 0.0)
                    nc.vector.memset(vpad[s][:], 0.0)

                for layer, (brx, bry) in enumerate([("a1", "a2"),
                                                    ("g1", "g2")]):
                    w_sb = {}
                    for br in (brx, bry):
                        for wn, dt_ in (("qw", BF16), ("kw", BF16),
                                        ("vw", BF16), ("pw", F32R)):
                            t = wts.tile([128, CK, C], dt_, tag="w_" + wn +
                                         ("_x" if br == brx else "_y"))
                            nc.sync.dma_start(
                                t[:], d[br + "_" + wn].rearrange(
                                    "k p c -> p k c"))
                            w_sb[br, wn] = t
                        for bn in ("qb", "kb", "pb"):
                            t = wts.tile([128, CK], F32, tag="w_" + bn +
                                         ("_x" if br == brx else "_y"))
                            nc.sync.dma_start(t[:], d[br + "_" + bn][:])
                            w_sb[br, bn] = t
                        t = wts.tile([1, C], BF16, tag="w_vb" +
                                     ("_x" if br == brx else "_y"))
                        nc.sync.dma_start(t[:], d[br + "_vb"][:])
                        w_sb[br, "vb"] = t

                    for g in range(NG):
                        _attn_group(nc, tc, d, scratch, X, w_sb, qg_sb,
                                    bt_sb, halfones, onesrow, kmask, vpad,
                                    attn, ps_m, ps_pvt, layer, brx, bry, g)

        # ---- phase D: residual + LN2 + MLP + residual ----
        tc.strict_bb_all_engine_barrier()
        with (
            tc.tile_pool(name="mlpw", bufs=1) as mlpw,
            tc.tile_pool(name="phd", bufs=2) as phd,
        ):
            mw = {}
            for m in ("m1", "m2"):
                t = mlpw.tile([128, CK, MLP_H], F32R, name=m + "w1")
                nc.sync.dma_start(t[:], d[m + "_w1"].rearrange("k p c -> p k c"))
                mw[m, "w1"] = t
                t = mlpw.tile([128, MHK, C], F32R, name=m + "w2")
                nc.sync.dma_start(t[:], d[m + "_w2"].rearrange("k p c -> p k c"))
                mw[m, "w2"] = t
                t = mlpw.tile([128, MHK], F32, name=m + "b1")
                nc.sync.dma_start(t[:], d[m + "_b1"][:])
                mw[m, "b1"] = t
                t = mlpw.tile([128, CK], F32, name=m + "b2")
                nc.sync.dma_start(t[:], d[m + "_b2"][:])
                mw[m, "b2"] = t

            for g in range(NG):
                for s, m, ln, outn in (("x", "m1", "n21", "xo"),
                                       ("y", "m2", "n22", "yo")):
                    _phase_d_group(nc, tc, d, scratch, lngb, mw, ident_f,
                                   eps, phd, ps_t, ps_m, s, m, ln, outn, g)



def _ln_stats(nc, pool, xr, eps, tagp):
    """mean/rstd over the free dim of xr [128, C] without BNStats (that
    instruction template only accepts one sync-wait on this toolchain)."""
    dummy = pool.tile([128, C], F32, tag=tagp + "_dm")
    sum_ = pool.tile([128, 1], F32, tag=tagp + "_sum")
    nc.vector.tensor_scalar(out=dummy[:], in0=xr[:], scalar1=0.0, scalar2=0.0,
                            op0=OP.add, op1=OP.add, accum_out=sum_[:])
    sumsq = pool.tile([128, 1], F32, tag=tagp + "_ssq")
    nc.vector.scalar_tensor_tensor(out=dummy[:], in0=xr[:], scalar=0.0,
                                   in1=xr[:], op0=OP.add, op1=OP.mult,
                                   accum_out=sumsq[:])
    mean = pool.tile([128, 1], F32, tag=tagp + "_mean")
    nc.vector.tensor_scalar(out=mean[:], in0=sum_[:], scalar1=1.0 / C,
                            scalar2=0.0, op0=OP.mult, op1=OP.add)
    m2 = pool.tile([128, 1], F32, tag=tagp + "_m2")
    nc.vector.tensor_mul(out=m2[:], in0=mean[:], in1=mean[:])
    var = pool.tile([128, 1], F32, tag=tagp + "_var")
    nc.vector.tensor_scalar(out=var[:], in0=sumsq[:], scalar1=1.0 / C,
                            scalar2=m2[:], op0=OP.mult, op1=OP.subtract)
    rstd = pool.tile([128, 1], F32, tag=tagp + "_rstd")
    nc.scalar.activation(out=rstd[:], in_=var[:], func=AF.Sqrt, bias=eps[:],
                         scale=1.0)
    nc.vector.reciprocal(out=rstd[:], in_=rstd[:])
    return mean, rstd


def _phase_a(nc, tc, d, lngb, ident_bf, eps, pha, ps_t, X):
    for s, ln in (("x", "n11"), ("y", "n12")):
        g_sb, b_sb = lngb[ln + "_g"], lngb[ln + "_b"]
        for g in range(NG):
            for tc4 in range(4):
                xr = pha.tile([128, C], F32, tag="pha_raw")
                _dma_win_chunk_in(nc, xr, d[s], g, tc4)
                mean, rstd = _ln_stats(nc, pha, xr, eps, "pha")
                xc = pha.tile([128, C], BF16, tag="pha_xc")
                nc.vector.tensor_scalar(
                    out=xc[:], in0=xr[:], scalar1=mean[:], scalar2=rstd[:],
                    op0=OP.subtract, op1=OP.mult)
                for c in range(CK):
                    pt = ps_t.tile([128, 128], BF16, tag="tp")
                    nc.tensor.transpose(pt[:], xc[:, c * 128:(c + 1) * 128],
                                        ident_bf[:])
                    nc.vector.tensor_scalar(
                        out=X[s][:, c, g * GT + tc4 * 128:
                                 g * GT + (tc4 + 1) * 128],
                        in0=pt[:], scalar1=g_sb[:, c:c + 1],
                        scalar2=b_sb[:, c:c + 1], op0=OP.mult, op1=OP.add)


def _attn_group(nc, tc, d, scratch, X, w_sb, qg_sb, bt_sb, halfones, onesrow,
                kmask, vpad, attn, ps_m, ps_pvt, layer, brx, bry, g):
    gsl = slice(g * GT, (g + 1) * GT)

    # fw = |X_g - Y_g|  (bf16), shared by both branches
    fw = attn.tile([128, CK, GT], BF16, tag="fw", bufs=1)
    nc.vector.tensor_sub(out=fw[:], in0=X["x"][:, :, gsl], in1=X["y"][:, :, gsl])
    nc.vector.scalar_tensor_tensor(
        out=fw[:], in0=fw[:], scalar=-1.0, in1=fw[:], op0=OP.mult, op1=OP.max)

    for s, br in (("x", brx), ("y", bry)):
        # ---- q projection ----
        q = attn.tile([128, CK, GT], BF16, tag="q_" + s)
        for mc in range(CK):
            pq = ps_m.tile([128, GT], F32, tag="mm")
            for kc in range(CK):
                nc.tensor.matmul(
                    pq[:], w_sb[br, "qw"][:, kc, mc * 128:(mc + 1) * 128],
                    X[s][:, kc, gsl], start=(kc == 0), stop=(kc == CK - 1))
            if layer == 0:
                nc.vector.tensor_scalar(
                    out=q[:, mc, :], in0=pq[:],
                    scalar1=w_sb[br, "qb"][:, mc:mc + 1], scalar2=0.0,
                    op0=OP.add, op1=OP.add)
            else:
                nc.vector.scalar_tensor_tensor(
                    out=q[:, mc, :].rearrange("p (w i) -> p w i", i=N),
                    in0=pq[:].rearrange("p (w i) -> p w i", i=N),
                    scalar=w_sb[br, "qb"][:, mc:mc + 1],
                    in1=qg_sb[s][:, mc, None, :].to_broadcast([128, GW, N]),
                    op0=OP.add, op1=OP.add)

        # ---- k projection into masked layout ----
        for mc in range(CK):
            pk = ps_m.tile([128, GT], F32, tag="mm")
            for kc in range(CK):
                nc.tensor.matmul(
                    pk[:], w_sb[br, "kw"][:, kc, mc * 128:(mc + 1) * 128],
                    fw[:, kc, :], start=(kc == 0), stop=(kc == CK - 1))
            for hs in range(4):
                rs = slice(hs * 32, (hs + 1) * 32)
                nc.vector.tensor_scalar(
                    out=kmask[s][rs, mc, hs, :], in0=pk[rs, :],
                    scalar1=w_sb[br, "kb"][rs, mc:mc + 1], scalar2=0.0,
                    op0=OP.add, op1=OP.add)

        # ---- v projection (activations stationary) into padded layout ----
        for tc4 in range(4):
            pv = ps_m.tile([128, C], F32, tag="mm")
            tsl = slice(g * GT + tc4 * 128, g * GT + (tc4 + 1) * 128)
            lsl = slice(tc4 * 128, (tc4 + 1) * 128)
            for kc in range(CK):
                nc.tensor.matmul(pv[:], fw[:, kc, lsl], w_sb[br, "vw"][:, kc, :],
                                 start=(kc == 0), stop=False)
            nc.tensor.matmul(pv[:], onesrow[:], w_sb[br, "vb"][:],
                             start=False, stop=True)
            vv = vpad[s].rearrange("p w h d -> p w (h d)")
            nc.vector.tensor_copy(out=vv[0:64, 2 * tc4, :], in_=pv[0:64, :])
            nc.vector.tensor_copy(out=vv[64:128, 2 * tc4 + 1, :],
                                  in_=pv[64:128, :])

        # ---- scores + softmax + PV per score-tile j ----
        pt_sb = attn.tile([128, 6, GT], BF16, tag="pt_" + s)
        rt_sb = attn.tile([128, 6, GT], BF16, tag="rt_" + s, bufs=1)
        pvt = [ps_pvt.tile([128, GT], F32, tag="pvt", name=f"pvt{i}")
               for i in range(CK)]

        for j in range(6):
            sc = ps_m.tile([128, GT], F32, tag="mm")
            for hh in range(2):
                h = 2 * j + hh
                for w in range(GW):
                    slot = (hh * 4 + w // 2) * N
                    wsl = slice(g * GT + w * N, g * GT + (w + 1) * N)
                    nc.tensor.matmul(
                        sc[(w % 2) * 64:(w % 2) * 64 + 64, slot:slot + N],
                        kmask[s][:, h // 4, h % 4,
                                 w * N:(w + 1) * N],
                        q[:, h // 4, w * N:(w + 1) * N],
                        start=True, stop=True,
                        tile_position=(0, (w % 2) * 64))
            # add relative-position bias (per-head, broadcast over w//2)
            nc.vector.tensor_tensor(
                out=sc[:].rearrange("p (hh ww i) -> p hh ww i", hh=2, i=N),
                in0=sc[:].rearrange("p (hh ww i) -> p hh ww i", hh=2, i=N),
                in1=bt_sb[br][:, 2 * j:2 * j + 2, None, :]
                .to_broadcast([128, 2, 4, N]),
                op=OP.add)
            nc.scalar.activation(out=pt_sb[:, j, :], in_=sc[:], func=AF.Exp)

            se = ps_m.tile([128, GT], F32, tag="mm")
            for hf in range(2):
                nc.tensor.matmul(
                    se[hf * 64:hf * 64 + 64, :], halfones[:, hf, :],
                    pt_sb[:, j, :], start=True, stop=True,
                    tile_position=(0, hf * 64))
            with nc.allow_low_precision(reason="softmax 1/sumexp in bf16"):
                nc.vector.reciprocal(out=rt_sb[:, j, :], in_=se[:])
        nc.vector.tensor_mul(out=pt_sb[:], in0=pt_sb[:], in1=rt_sb[:])

        # ---- P @ V (masked stationary v) ----
        for h in range(NH):
            j = h // 2
            hh = h % 2
            for w in range(GW):
                slot = (hh * 4 + w // 2) * N
                nc.tensor.matmul(
                    pvt[h // 4][(h % 4) * 32:(h % 4) * 32 + 32,
                                w * N:(w + 1) * N],
                    vpad[s][:, w, h, :],
                    pt_sb[:, j, slot:slot + N],
                    start=True, stop=True,
                    tile_position=(0, (h % 4) * 32))

        o_fm = attn.tile([128, CK, GT], F32R, tag="o_" + s, bufs=1)
        for t in range(CK):
            nc.scalar.activation(out=o_fm[:, t, :], in_=pvt[t][:], func=AF.Copy)

        # ---- out projection (fp32r) ----
        for mc in range(CK):
            po = ps_m.tile([128, GT], F32, tag="mm")
            for kc in range(CK):
                nc.tensor.matmul(
                    po[:],
                    w_sb[br, "pw"][:, kc, mc * 128:(mc + 1) * 128],
                    o_fm[:, kc, :],
                    start=(kc == 0), stop=(kc == CK - 1))
            if layer == 0:
                nc.vector.tensor_scalar(
                    out=X[s][:, mc, gsl], in0=po[:],
                    scalar1=w_sb[br, "pb"][:, mc:mc + 1], scalar2=0.0,
                    op0=OP.add, op1=OP.add)
            else:
                x2 = attn.tile([128, GT], F32, tag="x2_" + s, bufs=1)
                nc.vector.tensor_scalar(
                    out=x2[:], in0=po[:],
                    scalar1=w_sb[br, "pb"][:, mc:mc + 1], scalar2=0.0,
                    op0=OP.add, op1=OP.add)
                nc.sync.dma_start(scratch[s][:, mc, gsl], x2[:])


def _phase_d_group(nc, tc, d, scratch, lngb, mw, ident_f, eps, phd, ps_t,
                   ps_m, s, m, ln, outn, g):
    gsl = slice(g * GT, (g + 1) * GT)
    g_sb, b_sb = lngb[ln + "_g"], lngb[ln + "_b"]

    x2sb = phd.tile([128, CK, GT], F32, tag="d_x2")
    nc.sync.dma_start(x2sb[:], scratch[s][:, :, gsl])

    xo_tm = phd.tile([128, 4, C], F32, tag="d_xo")
    for tc4 in range(4):
        xr = phd.tile([128, C], F32, tag="d_raw")
        _dma_win_chunk_in(nc, xr, d[s], g, tc4)
        for c in range(CK):
            pt = ps_t.tile([128, 128], F32, tag="tp")
            nc.tensor.transpose(pt[:], x2sb[:, c, tc4 * 128:(tc4 + 1) * 128],
                                ident_f[:])
            nc.vector.tensor_add(out=xo_tm[:, tc4, c * 128:(c + 1) * 128],
                                 in0=pt[:], in1=xr[:, c * 128:(c + 1) * 128])

    ln2 = phd.tile([128, CK, GT], F32R, tag="d_ln2")
    for tc4 in range(4):
        mean, rstd = _ln_stats(nc, phd, xo_tm[:, tc4, :], eps, "d")
        xc = phd.tile([128, C], F32, tag="d_xc")
        nc.vector.tensor_scalar(
            out=xc[:], in0=xo_tm[:, tc4, :], scalar1=mean[:],
            scalar2=rstd[:], op0=OP.subtract, op1=OP.mult)
        for c in range(CK):
            pt = ps_t.tile([128, 128], F32, tag="tp")
            nc.tensor.transpose(pt[:], xc[:, c * 128:(c + 1) * 128], ident_f[:])
            nc.vector.tensor_scalar(
                out=ln2[:, c, tc4 * 128:(tc4 + 1) * 128], in0=pt[:],
                scalar1=g_sb[:, c:c + 1], scalar2=b_sb[:, c:c + 1],
                op0=OP.mult, op1=OP.add)

    # MLP
    h_fm = phd.tile([128, MHK, GT], F32R, tag="d_h")
    for mc in range(MHK):
        ph = ps_m.tile([128, GT], F32, tag="mm")
        for kc in range(CK):
            nc.tensor.matmul(
                ph[:], mw[m, "w1"][:, kc, mc * 128:(mc + 1) * 128],
                ln2[:, kc, :],
                start=(kc == 0), stop=(kc == CK - 1))
        nc.scalar.activation(out=h_fm[:, mc, :], in_=ph[:], func=AF.Gelu,
                             bias=mw[m, "b1"][:, mc:mc + 1], scale=1.0)

    mlp_fm = phd.tile([128, CK, GT], F32, tag="d_mlp")
    for mc in range(CK):
        po = ps_m.tile([128, GT], F32, tag="mm")
        for kc in range(MHK):
            nc.tensor.matmul(
                po[:], mw[m, "w2"][:, kc, mc * 128:(mc + 1) * 128],
                h_fm[:, kc, :],
                start=(kc == 0), stop=(kc == MHK - 1))
        nc.vector.tensor_scalar(
            out=mlp_fm[:, mc, :], in0=po[:], scalar1=mw[m, "b2"][:, mc:mc + 1],
            scalar2=0.0, op0=OP.add, op1=OP.add)

    for tc4 in range(4):
        ot = phd.tile([128, C], F32, tag="d_out")
        for c in range(CK):
            pt = ps_t.tile([128, 128], F32, tag="tp")
            nc.tensor.transpose(pt[:], mlp_fm[:, c, tc4 * 128:(tc4 + 1) * 128],
                                ident_f[:])
            nc.vector.tensor_add(out=ot[:, c * 128:(c + 1) * 128], in0=pt[:],
                                 in1=xo_tm[:, tc4, c * 128:(c + 1) * 128])
        _dma_win_chunk_out(nc, d[outn], ot, g, tc4)


# ---------------------------------------------------------------------------
# host side
# ---------------------------------------------------------------------------

_NC_CACHE = None


def _rel_pos_index(ws):
    coords = np.stack(np.meshgrid(np.arange(ws), np.arange(ws),
                                  indexing="ij")).reshape(2, -1)
    rel = (coords[:, :, None] - coords[:, None, :]).transpose(1, 2, 0).copy()
    rel[:, :, 0] += ws - 1
    rel[:, :, 1] += ws - 1
    rel[:, :, 0] *= 2 * ws - 1
    return rel.sum(-1)


def _prep_shared(inputs):
    """Host-side layout prep of weights (shared across cores)."""
    import ml_dtypes
    bf = ml_dtypes.bfloat16
    sh = {}
    rpi = _rel_pos_index(WS)
    for br in BRANCHES:
        qw = inputs[br + "_qw"]
        kvw = inputs[br + "_kvw"]
        kvb = inputs[br + "_kvb"]
        sh[br + "_qw"] = np.ascontiguousarray(
            qw.reshape(CK, 128, C)).astype(bf)
        sh[br + "_kw"] = np.ascontiguousarray(
            (kvw[:, :C] * SCALE).reshape(CK, 128, C)).astype(bf)
        sh[br + "_vw"] = np.ascontiguousarray(
            kvw[:, C:].reshape(CK, 128, C)).astype(bf)
        sh[br + "_pw"] = np.ascontiguousarray(
            inputs[br + "_pw"].reshape(CK, 128, C)).astype(np.float32)
        sh[br + "_qb"] = np.ascontiguousarray(
            inputs[br + "_qb"].reshape(CK, 128).T).astype(np.float32)
        sh[br + "_kb"] = np.ascontiguousarray(
            (kvb[:C] * SCALE).reshape(CK, 128).T).astype(np.float32)
        sh[br + "_vb"] = kvb[C:].reshape(1, C).astype(bf)
        sh[br + "_pb"] = np.ascontiguousarray(
            inputs[br + "_pb"].reshape(CK, 128).T).astype(np.float32)
        # rel-pos bias, transposed layout [tk, h, tq], duplicated across halves
        bfull = np.asarray(inputs[br + "_rpb"])[rpi]       # [n, m, NH]
        btr = np.transpose(bfull, (1, 2, 0))               # [m(tk), NH, n(tq)]
        sh[br + "_bt"] = np.concatenate([btr, btr], axis=0).astype(np.float32)
    for m in ("m1", "m2"):
        sh[m + "_w1"] = np.ascontiguousarray(
            inputs[m + "_w1"].reshape(CK, 128, MLP_H)).astype(np.float32)
        sh[m + "_b1"] = np.ascontiguousarray(
            inputs[m + "_b1"].reshape(MHK, 128).T).astype(np.float32)
        sh[m + "_w2"] = np.ascontiguousarray(
            inputs[m + "_w2"].reshape(MHK, 128, C)).astype(np.float32)
        sh[m + "_b2"] = np.ascontiguousarray(
            inputs[m + "_b2"].reshape(CK, 128).T).astype(np.float32)
    for ln in ("n11", "n12", "n21", "n22"):
        sh[ln + "_g"] = np.ascontiguousarray(
            inputs[ln + "_g"].reshape(CK, 128).T).astype(np.float32)
        sh[ln + "_b"] = np.ascontiguousarray(
            inputs[ln + "_b"].reshape(CK, 128).T).astype(np.float32)
    sh["ident_bf"] = np.eye(128, dtype=bf)
    sh["ident_f32"] = np.eye(128, dtype=np.float32)
    ho = np.zeros((128, 2, 64), dtype=bf)
    ho[0:64, 0, :] = 1
    ho[64:128, 1, :] = 1
    sh["halfones"] = ho
    sh["onesrow"] = np.ones((1, 128), dtype=bf)
    return sh


def kernel(**inputs):
    global _NC_CACHE
    if _NC_CACHE is None:
        _NC_CACHE = build_nc()
    nc = _NC_CACHE

    sh = _prep_shared(inputs)
    in_maps = []
    for b in range(B):
        im = dict(sh)
        im["x"] = np.ascontiguousarray(inputs["x"][b]).astype(np.float32)
        im["y"] = np.ascontiguousarray(inputs["y"][b]).astype(np.float32)
        for s in ("x", "y"):
            qg = np.asarray(inputs[s + "_q"])[b, 0]        # [NH, N, HD]
            qfm = qg.transpose(0, 2, 1).reshape(C, N)      # [(h d), n]
            im["qg_" + s] = np.ascontiguousarray(
                qfm.reshape(CK, 128, N).transpose(1, 0, 2)).astype(np.float32)
        in_maps.append(im)

    res = run_bass_kernel_spmd(nc, in_maps, core_ids=list(range(B)))
    xo = np.stack([res.results[b]["xo"] for b in range(B)])
    yo = np.stack([res.results[b]["yo"] for b in range(B)])
    return xo.astype(np.float32), yo.astype(np.float32)


# revision 27
# speedup vs baseline: 4079.9545x; 4079.9545x over previous
"""DiffFormerBlock Trainium2 kernel.

Data-parallel over batch B=8: core b processes image b end-to-end (no
collectives).  Per-core pipeline, all activations feature-major ("fm",
[C-chunk partitions, tokens]) inside the attention block:

  phase A: LN(x), LN(y) token-major (bn_stats) -> prescale -> PE-transpose
           -> gamma/beta -> X_fm, Y_fm (bf16, window-ordered tokens)
  phase B (layer 1) / phase C (layer 2), per 8-window group, per branch:
           q/k projections (bf16 matmul), v projection (activations
           stationary -> token-major v), scores S^T = k^T q per (head,
           window) with half-masked stationaries (no row tiling), bias add
           on PSUM, exp on ACT, sum-exp via half-ones matmul, reciprocal,
           normalize P^T, P@V via masked v -> O^T feature-major, out-proj
           (fp32r).  Layer 1 writes back into X_fm/Y_fm, layer 2 spills
           fp32 x2/y2 to DRAM scratch.
  phase D: residual + LN2 + MLP (fp32r) + residual, transposes via PE,
           final store in raster order.
"""

import numpy as np

import concourse.bass as bass
import concourse.tile as tile
import concourse.mybir as mybir
from concourse.bass_utils import run_bass_kernel_spmd
from concourse.vector_clock import ScopedClock

F32 = mybir.dt.float32
F32R = mybir.dt.float32r
BF16 = mybir.dt.bfloat16
AF = mybir.ActivationFunctionType
OP = mybir.AluOpType

B = 8
HW = 64            # image height == width
C = 384
CK = 3             # C / 128
NH = 12
HD = 32
WS = 8             # window size
N = 64             # tokens per window
T = 4096           # tokens per image
NWIN = 64          # windows per image
NG = 8             # window groups (one row of 8 windows each)
GW = 8             # windows per group
GT = 512           # tokens per group
MLP_H = 1536
MHK = 12           # MLP_H / 128
SCALE = HD ** -0.5


def _split_excess_waits(nc):
    """The pinned walrus accepts exactly ONE sync-wait command per
    instruction (any opcode).  Hoist surplus waits onto single-wait NoOps
    inserted just before the instruction on the same engine — the engine
    sequencer executes them in order, so the downstream instruction still
    starts only after every semaphore condition is met."""
    ctr = [0]
    for fn in nc.m.functions:
        for blk in fn.blocks:
            out = []
            changed = False
            for inst in blk.instructions:
                si = getattr(inst, "sync_info", None)
                ow = list(si.on_wait) if (si is not None and si.on_wait) else []
                if len(ow) > 1:
                    for w in ow[:-1]:
                        ctr[0] += 1
                        nop = mybir.InstNoOp(
                            name=f"waitnop-{ctr[0]}", ins=[], outs=[])
                        nop.engine = inst.engine
                        nop.sync_info = mybir.SyncInfo(on_wait=[w],
                                                       on_update=[])
                        out.append(nop)
                    inst.sync_info = mybir.SyncInfo(
                        on_wait=[ow[-1]],
                        on_update=list(si.on_update or []))
                    changed = True
                out.append(inst)
            if changed:
                blk.instructions = out


class _PatchedTileContext(tile.TileContext):
    """The pinned walrus accepts at most one sync-wait on a Drain; spread the
    exit drain's extra DMA-queue waits across individual SP nops."""

    def schedule_and_allocate(self):
        res = super().schedule_and_allocate()
        _split_excess_waits(self.nc)
        return res

    def _drain_and_barrier(self, tick_clock, wait_clock):
        drain_inst = self.nc.sync.drain()
        wait_clock.add_sem_waits(
            drain_inst.ins, ScopedClock({None: tick_clock.global_clock})
        )
        si = drain_inst.ins.sync_info
        if si is not None and si.on_wait is not None and len(si.on_wait) > 1:
            waits = list(si.on_wait)
            drain_inst.ins.sync_info = mybir.SyncInfo(
                on_wait=[waits[0]], on_update=list(si.on_update or [])
            )
            for w in waits[1:]:
                nop = self.nc.sync.nop(nofuse=True)
                nop.ins.sync_info = mybir.SyncInfo(on_wait=[w], on_update=[])

        self.nc.all_engine_barrier()
        assert self.sems is not None
        popped = self.nc._tile_sem_poison_stack.pop()
        assert popped is self._sem_poison
        self.nc.clear_and_free_semaphores(list(self.sems.allocated().values()))
        self.nc.all_engine_barrier()


def _win_view(dram_ap):
    """[T, C] raster -> [wh, ih, ww, iw, C]."""
    return dram_ap.rearrange(
        "(wh ih ww iw) c -> wh ih ww iw c", wh=WS, ih=WS, ww=WS, iw=WS
    )


def _win_half(dram_ap, g, tc4, ww):
    """AP for one window (64 tokens) of chunk (g, tc4): [ih, iw, C]."""
    v = _win_view(dram_ap)
    return v[g, :, 2 * tc4 + ww, :, :]


def _dma_win_chunk_in(nc, xr, dram_ap, g, tc4):
    for ww in range(2):
        nc.sync.dma_start(xr[ww * 64:(ww + 1) * 64, :],
                          _win_half(dram_ap, g, tc4, ww))


def _dma_win_chunk_out(nc, dram_ap, ot, g, tc4):
    for ww in range(2):
        nc.sync.dma_start(_win_half(dram_ap, g, tc4, ww),
                          ot[ww * 64:(ww + 1) * 64, :])


BRANCHES = ["a1", "a2", "g1", "g2"]


def build_nc():
    nc = bass.Bass()

    # ---- DRAM I/O ----
    d = {}
    for s in ("x", "y"):
        d[s] = nc.dram_tensor(s, [T, C], F32, kind="ExternalInput")
        d["qg_" + s] = nc.dram_tensor("qg_" + s, [128, CK, N], F32,
                                      kind="ExternalInput")
    for br in BRANCHES:
        d[br + "_qw"] = nc.dram_tensor(br + "_qw", [CK, 128, C], BF16,
                                       kind="ExternalInput")
        d[br + "_kw"] = nc.dram_tensor(br + "_kw", [CK, 128, C], BF16,
                                       kind="ExternalInput")
        d[br + "_vw"] = nc.dram_tensor(br + "_vw", [CK, 128, C], BF16,
                                       kind="ExternalInput")
        d[br + "_pw"] = nc.dram_tensor(br + "_pw", [CK, 128, C], F32R,
                                       kind="ExternalInput")
        d[br + "_qb"] = nc.dram_tensor(br + "_qb", [128, CK], F32,
                                       kind="ExternalInput")
        d[br + "_kb"] = nc.dram_tensor(br + "_kb", [128, CK], F32,
                                       kind="ExternalInput")
        d[br + "_vb"] = nc.dram_tensor(br + "_vb", [1, C], BF16,
                                       kind="ExternalInput")
        d[br + "_pb"] = nc.dram_tensor(br + "_pb", [128, CK], F32,
                                       kind="ExternalInput")
        d[br + "_bt"] = nc.dram_tensor(br + "_bt", [128, NH, N], F32,
                                       kind="ExternalInput")
    for m in ("m1", "m2"):
        d[m + "_w1"] = nc.dram_tensor(m + "_w1", [CK, 128, MLP_H], F32R,
                                      kind="ExternalInput")
        d[m + "_b1"] = nc.dram_tensor(m + "_b1", [128, MHK], F32,
                                      kind="ExternalInput")
        d[m + "_w2"] = nc.dram_tensor(m + "_w2", [MHK, 128, C], F32R,
                                      kind="ExternalInput")
        d[m + "_b2"] = nc.dram_tensor(m + "_b2", [128, CK], F32,
                                      kind="ExternalInput")
    for ln in ("n11", "n12", "n21", "n22"):
        d[ln + "_g"] = nc.dram_tensor(ln + "_g", [128, CK], F32,
                                      kind="ExternalInput")
        d[ln + "_b"] = nc.dram_tensor(ln + "_b", [128, CK], F32,
                                      kind="ExternalInput")
    d["ident_bf"] = nc.dram_tensor("ident_bf", [128, 128], BF16,
                                   kind="ExternalInput")
    d["ident_f32"] = nc.dram_tensor("ident_f32", [128, 128], F32,
                                    kind="ExternalInput")
    d["halfones"] = nc.dram_tensor("halfones", [128, 2, 64], BF16,
                                   kind="ExternalInput")
    d["onesrow"] = nc.dram_tensor("onesrow", [1, 128], BF16,
                                  kind="ExternalInput")

    d["xo"] = nc.dram_tensor("xo", [T, C], F32, kind="ExternalOutput")
    d["yo"] = nc.dram_tensor("yo", [T, C], F32, kind="ExternalOutput")

    # DRAM scratch for layer-2 outputs (feature-major fp32)
    scratch = {
        "x": nc.dram_tensor("x2s", [128, CK, T], F32, kind="Internal"),
        "y": nc.dram_tensor("y2s", [128, CK, T], F32, kind="Internal"),
    }

    with _PatchedTileContext(nc) as tc:
        _emit(nc, tc, d, scratch)
    return nc


def _emit(nc, tc, d, scratch):
    import contextlib

    ctx = contextlib.ExitStack()
    with ctx:
        persist = ctx.enter_context(tc.tile_pool(name="persist", bufs=1))
        # psum pools (<= 8 banks total while attention pools are open)
        ps_t = ctx.enter_context(
            tc.tile_pool(name="ps_t", bufs=2, space="PSUM"))     # transposes
        ps_m = ctx.enter_context(
            tc.tile_pool(name="ps_m", bufs=3, space="PSUM"))     # proj/scores/SE

        # ---- persistent constants ----
        ident_bf = persist.tile([128, 128], BF16)
        nc.sync.dma_start(ident_bf[:], d["ident_bf"][:])
        ident_f = persist.tile([128, 128], F32)
        nc.sync.dma_start(ident_f[:], d["ident_f32"][:])
        halfones = persist.tile([128, 2, 64], BF16)
        nc.sync.dma_start(halfones[:], d["halfones"][:])
        onesrow = persist.tile([1, 128], BF16)
        nc.sync.dma_start(onesrow[:], d["onesrow"][:])
        eps = persist.tile([128, 1], F32)
        nc.vector.memset(eps[:], 1e-5)

        lngb = {}
        for ln in ("n11", "n12", "n21", "n22"):
            for gb in ("_g", "_b"):
                t = persist.tile([128, CK], F32, name=ln + gb)
                nc.sync.dma_start(t[:], d[ln + gb][:])
                lngb[ln + gb] = t
        qg_sb = {}
        for s in ("x", "y"):
            t = persist.tile([128, CK, N], F32, name="qg" + s)
            nc.sync.dma_start(t[:], d["qg_" + s][:])
            qg_sb[s] = t

        with tc.tile_pool(name="xy", bufs=1) as xyp:
            X = {
                "x": xyp.tile([128, CK, T], BF16, name="X_fm"),
                "y": xyp.tile([128, CK, T], BF16, name="Y_fm"),
            }
            with tc.tile_pool(name="pha", bufs=3) as pha:
                _phase_a(nc, tc, d, lngb, ident_bf, eps, pha, ps_t, X)
            # pool space is reused across phases; hard-fence the transitions
            tc.strict_bb_all_engine_barrier()
            with (
                tc.tile_pool(name="wts", bufs=1) as wts,
                tc.tile_pool(name="attn", bufs=2) as attn,
                tc.tile_pool(name="attn1", bufs=1) as attn1,
                tc.tile_pool(name="ps_pvt", bufs=3, space="PSUM") as ps_pvt,
            ):
                # one-time zero backgrounds for masked stationaries
                kmask = {
                    s: attn1.tile([128, CK, 4, GT], BF16, name=f"kmask{s}")
                    for s in ("x", "y")
                }
                vpad = {
                    s: attn1.tile([128, GW, NH, HD], BF16, name=f"vpad{s}")
                    for s in ("x", "y")
                }
                for t in kmask.values():
                    nc.vector.memset(t[:], 0.0)
                for t in vpad.values():
                    nc.vector.memset(t[:], 0.0)

                for layer, (brx, bry) in enumerate([("a1", "a2"),
                                                    ("g1", "g2")]):
                    w_sb = {}
                    for br in (brx, bry):
                        for wn, dt_ in (("qw", BF16), ("kw", BF16),
                                        ("vw", BF16), ("pw", F32R)):
                            t = wts.tile([128, CK, C], dt_, tag="w_" + wn +
                                         ("_x" if br == brx else "_y"))
                            nc.sync.dma_start(
                                t[:], d[br + "_" + wn].rearrange(
                                    "k p c -> p k c"))
                            w_sb[br, wn] = t
                        for bn in ("qb", "kb", "pb"):
                            t = wts.tile([128, CK], F32, tag="w_" + bn +
                                         ("_x" if br == brx else "_y"))
                            nc.sync.dma_start(t[:], d[br + "_" + bn][:])
                            w_sb[br, bn] = t
                        t = wts.tile([1, C], BF16, tag="w_vb" +
                                     ("_x" if br == brx else "_y"))
                        nc.sync.dma_start(t[:], d[br + "_vb"][:])
                        w_sb[br, "vb"] = t
                        t = wts.tile([128, NH, N], F32, tag="w_bt" +
                                     ("_x" if br == brx else "_y"))
                        nc.sync.dma_start(t[:], d[br + "_bt"][:])
                        w_sb[br, "bt"] = t

                    for g in range(NG):
                        _attn_group(nc, tc, d, scratch, X, w_sb, qg_sb,
                                    halfones, onesrow, kmask, vpad,
                                    attn, ps_m, ps_pvt, layer, brx, bry, g)

        # ---- phase D: residual + LN2 + MLP + residual ----
        tc.strict_bb_all_engine_barrier()
        with (
            tc.tile_pool(name="mlpw", bufs=1) as mlpw,
            tc.tile_pool(name="phd", bufs=2) as phd,
        ):
            mw = {}
            for m in ("m1", "m2"):
                t = mlpw.tile([128, CK, MLP_H], F32R, name=m + "w1")
                nc.sync.dma_start(t[:], d[m + "_w1"].rearrange("k p c -> p k c"))
                mw[m, "w1"] = t
                t = mlpw.tile([128, MHK, C], F32R, name=m + "w2")
                nc.sync.dma_start(t[:], d[m + "_w2"].rearrange("k p c -> p k c"))
                mw[m, "w2"] = t
                t = mlpw.tile([128, MHK], F32, name=m + "b1")
                nc.sync.dma_start(t[:], d[m + "_b1"][:])
                mw[m, "b1"] = t
                t = mlpw.tile([128, CK], F32, name=m + "b2")
                nc.sync.dma_start(t[:], d[m + "_b2"][:])
                mw[m, "b2"] = t

            for g in range(NG):
                for s, m, ln, outn in (("x", "m1", "n21", "xo"),
                                       ("y", "m2", "n22", "yo")):
                    _phase_d_group(nc, tc, d, scratch, lngb, mw, ident_f,
                                   eps, phd, ps_t, ps_m, s, m, ln, outn, g)



def _ln_stats(nc, pool, xr, eps, tagp):
    """mean/rstd over the free dim of xr [128, C] without BNStats (that
    instruction template only accepts one sync-wait on this toolchain)."""
    dummy = pool.tile([128, C], F32, tag=tagp + "_dm")
    sum_ = pool.tile([128, 1], F32, tag=tagp + "_sum")
    nc.vector.tensor_scalar(out=dummy[:], in0=xr[:], scalar1=0.0, scalar2=0.0,
                            op0=OP.add, op1=OP.add, accum_out=sum_[:])
    sumsq = pool.tile([128, 1], F32, tag=tagp + "_ssq")
    nc.vector.scalar_tensor_tensor(out=dummy[:], in0=xr[:], scalar=0.0,
                                   in1=xr[:], op0=OP.add, op1=OP.mult,
                                   accum_out=sumsq[:])
    mean = pool.tile([128, 1], F32, tag=tagp + "_mean")
    nc.vector.tensor_scalar(out=mean[:], in0=sum_[:], scalar1=1.0 / C,
                            scalar2=0.0, op0=OP.mult, op1=OP.add)
    m2 = pool.tile([128, 1], F32, tag=tagp + "_m2")
    nc.vector.tensor_mul(out=m2[:], in0=mean[:], in1=mean[:])
    var = pool.tile([128, 1], F32, tag=tagp + "_var")
    nc.vector.tensor_scalar(out=var[:], in0=sumsq[:], scalar1=1.0 / C,
                            scalar2=m2[:], op0=OP.mult, op1=OP.subtract)
    rstd = pool.tile([128, 1], F32, tag=tagp + "_rstd")
    nc.scalar.activation(out=rstd[:], in_=var[:], func=AF.Sqrt, bias=eps[:],
                         scale=1.0)
    nc.vector.reciprocal(out=rstd[:], in_=rstd[:])
    return mean, rstd


def _phase_a(nc, tc, d, lngb, ident_bf, eps, pha, ps_t, X):
    for s, ln in (("x", "n11"), ("y", "n12")):
        g_sb, b_sb = lngb[ln + "_g"], lngb[ln + "_b"]
        for g in range(NG):
            for tc4 in range(4):
                xr = pha.tile([128, C], F32, tag="pha_raw")
                _dma_win_chunk_in(nc, xr, d[s], g, tc4)
                mean, rstd = _ln_stats(nc, pha, xr, eps, "pha")
                xc = pha.tile([128, C], BF16, tag="pha_xc")
                nc.vector.tensor_scalar(
                    out=xc[:], in0=xr[:], scalar1=mean[:], scalar2=rstd[:],
                    op0=OP.subtract, op1=OP.mult)
                for c in range(CK):
                    pt = ps_t.tile([128, 128], BF16, tag="tp")
                    nc.tensor.transpose(pt[:], xc[:, c * 128:(c + 1) * 128],
                                        ident_bf[:])
                    nc.vector.tensor_scalar(
                        out=X[s][:, c, g * GT + tc4 * 128:
                                 g * GT + (tc4 + 1) * 128],
                        in0=pt[:], scalar1=g_sb[:, c:c + 1],
                        scalar2=b_sb[:, c:c + 1], op0=OP.mult, op1=OP.add)


def _attn_group(nc, tc, d, scratch, X, w_sb, qg_sb, halfones, onesrow,
                kmask, vpad, attn, ps_m, ps_pvt, layer, brx, bry, g):
    gsl = slice(g * GT, (g + 1) * GT)

    # fw = |X_g - Y_g|  (bf16), shared by both branches
    fw = attn.tile([128, CK, GT], BF16, tag="fw", bufs=1)
    nc.vector.tensor_sub(out=fw[:], in0=X["x"][:, :, gsl], in1=X["y"][:, :, gsl])
    nc.vector.scalar_tensor_tensor(
        out=fw[:], in0=fw[:], scalar=-1.0, in1=fw[:], op0=OP.mult, op1=OP.max)

    for s, br in (("x", brx), ("y", bry)):
        # ---- q projection ----
        q = attn.tile([128, CK, GT], BF16, tag="q_" + s)
        for mc in range(CK):
            pq = ps_m.tile([128, GT], F32, tag="mm")
            for kc in range(CK):
                nc.tensor.matmul(
                    pq[:], w_sb[br, "qw"][:, kc, mc * 128:(mc + 1) * 128],
                    X[s][:, kc, gsl], start=(kc == 0), stop=(kc == CK - 1))
            if layer == 0:
                nc.vector.tensor_scalar(
                    out=q[:, mc, :], in0=pq[:],
                    scalar1=w_sb[br, "qb"][:, mc:mc + 1], scalar2=0.0,
                    op0=OP.add, op1=OP.add)
            else:
                nc.vector.scalar_tensor_tensor(
                    out=q[:, mc, :].rearrange("p (w i) -> p w i", i=N),
                    in0=pq[:].rearrange("p (w i) -> p w i", i=N),
                    scalar=w_sb[br, "qb"][:, mc:mc + 1],
                    in1=qg_sb[s][:, mc, None, :].to_broadcast([128, GW, N]),
                    op0=OP.add, op1=OP.add)

        # ---- k projection into masked layout ----
        for mc in range(CK):
            pk = ps_m.tile([128, GT], F32, tag="mm")
            for kc in range(CK):
                nc.tensor.matmul(
                    pk[:], w_sb[br, "kw"][:, kc, mc * 128:(mc + 1) * 128],
                    fw[:, kc, :], start=(kc == 0), stop=(kc == CK - 1))
            for hs in range(4):
                rs = slice(hs * 32, (hs + 1) * 32)
                nc.vector.tensor_scalar(
                    out=kmask[s][rs, mc, hs, :], in0=pk[rs, :],
                    scalar1=w_sb[br, "kb"][rs, mc:mc + 1], scalar2=0.0,
                    op0=OP.add, op1=OP.add)

        # ---- v projection (activations stationary) into padded layout ----
        for tc4 in range(4):
            pv = ps_m.tile([128, C], F32, tag="mm")
            tsl = slice(g * GT + tc4 * 128, g * GT + (tc4 + 1) * 128)
            lsl = slice(tc4 * 128, (tc4 + 1) * 128)
            for kc in range(CK):
                nc.tensor.matmul(pv[:], fw[:, kc, lsl], w_sb[br, "vw"][:, kc, :],
                                 start=(kc == 0), stop=False)
            nc.tensor.matmul(pv[:], onesrow[:], w_sb[br, "vb"][:],
                             start=False, stop=True)
            vv = vpad[s].rearrange("p w h d -> p w (h d)")
            nc.vector.tensor_copy(out=vv[0:64, 2 * tc4, :], in_=pv[0:64, :])
            nc.vector.tensor_copy(out=vv[64:128, 2 * tc4 + 1, :],
                                  in_=pv[64:128, :])

        # ---- scores + softmax + PV per score-tile j ----
        pt_sb = attn.tile([128, 6, GT], BF16, tag="pt_" + s)
        rt_sb = attn.tile([128, 6, GT], BF16, tag="rt_" + s, bufs=1)
        pvt = [ps_pvt.tile([128, GT], F32, tag="pvt", name=f"pvt{i}")
               for i in range(CK)]

        for j in range(6):
            sc = ps_m.tile([128, GT], F32, tag="mm")
            for hh in range(2):
                h = 2 * j + hh
                for w in range(GW):
                    slot = (hh * 4 + w // 2) * N
                    wsl = slice(g * GT + w * N, g * GT + (w + 1) * N)
                    nc.tensor.matmul(
                        sc[(w % 2) * 64:(w % 2) * 64 + 64, slot:slot + N],
                        kmask[s][:, h // 4, h % 4,
                                 w * N:(w + 1) * N],
                        q[:, h // 4, w * N:(w + 1) * N],
                        start=True, stop=True,
                        tile_position=(0, (w % 2) * 64))
            # add relative-position bias (per-head, broadcast over w//2)
            nc.vector.tensor_tensor(
                out=sc[:].rearrange("p (hh ww i) -> p hh ww i", hh=2, i=N),
                in0=sc[:].rearrange("p (hh ww i) -> p hh ww i", hh=2, i=N),
                in1=w_sb[br, "bt"][:, 2 * j:2 * j + 2, None, :]
                .to_broadcast([128, 2, 4, N]),
                op=OP.add)
            nc.scalar.activation(out=pt_sb[:, j, :], in_=sc[:], func=AF.Exp)

            se = ps_m.tile([128, GT], F32, tag="mm")
            for hf in range(2):
                nc.tensor.matmul(
                    se[hf * 64:hf * 64 + 64, :], halfones[:, hf, :],
                    pt_sb[:, j, :], start=True, stop=True,
                    tile_position=(0, hf * 64))
            with nc.allow_low_precision(reason="softmax 1/sumexp in bf16"):
                nc.vector.reciprocal(out=rt_sb[:, j, :], in_=se[:])
        nc.vector.tensor_mul(out=pt_sb[:], in0=pt_sb[:], in1=rt_sb[:])

        # ---- P @ V (masked stationary v) ----
        for h in range(NH):
            j = h // 2
            hh = h % 2
            for w in range(GW):
                slot = (hh * 4 + w // 2) * N
                nc.tensor.matmul(
                    pvt[h // 4][(h % 4) * 32:(h % 4) * 32 + 32,
                                w * N:(w + 1) * N],
                    vpad[s][:, w, h, :],
                    pt_sb[:, j, slot:slot + N],
                    start=True, stop=True,
                    tile_position=(0, (h % 4) * 32))

        o_fm = attn.tile([128, CK, GT], F32R, tag="o_" + s, bufs=1)
        for t in range(CK):
            nc.scalar.activation(out=o_fm[:, t, :], in_=pvt[t][:], func=AF.Copy)

        # ---- out projection (fp32r) ----
        for mc in range(CK):
            po = ps_m.tile([128, GT], F32, tag="mm")
            for kc in range(CK):
                nc.tensor.matmul(
                    po[:],
                    w_sb[br, "pw"][:, kc, mc * 128:(mc + 1) * 128],
                    o_fm[:, kc, :],
                    start=(kc == 0), stop=(kc == CK - 1))
            if layer == 0:
                nc.vector.tensor_scalar(
                    out=X[s][:, mc, gsl], in0=po[:],
                    scalar1=w_sb[br, "pb"][:, mc:mc + 1], scalar2=0.0,
                    op0=OP.add, op1=OP.add)
            else:
                x2 = attn.tile([128, GT], F32, tag="x2_" + s, bufs=1)
                nc.vector.tensor_scalar(
                    out=x2[:], in0=po[:],
                    scalar1=w_sb[br, "pb"][:, mc:mc + 1], scalar2=0.0,
                    op0=OP.add, op1=OP.add)
                nc.sync.dma_start(scratch[s][:, mc, gsl], x2[:])


def _phase_d_group(nc, tc, d, scratch, lngb, mw, ident_f, eps, phd, ps_t,
                   ps_m, s, m, ln, outn, g):
    gsl = slice(g * GT, (g + 1) * GT)
    g_sb, b_sb = lngb[ln + "_g"], lngb[ln + "_b"]

    x2sb = phd.tile([128, CK, GT], F32, tag="d_x2")
    nc.sync.dma_start(x2sb[:], scratch[s][:, :, gsl])

    xo_tm = phd.tile([128, 4, C], F32, tag="d_xo")
    for tc4 in range(4):
        xr = phd.tile([128, C], F32, tag="d_raw")
        _dma_win_chunk_in(nc, xr, d[s], g, tc4)
        for c in range(CK):
            pt = ps_t.tile([128, 128], F32, tag="tp")
            nc.tensor.transpose(pt[:], x2sb[:, c, tc4 * 128:(tc4 + 1) * 128],
                                ident_f[:])
            nc.vector.tensor_add(out=xo_tm[:, tc4, c * 128:(c + 1) * 128],
                                 in0=pt[:], in1=xr[:, c * 128:(c + 1) * 128])

    ln2 = phd.tile([128, CK, GT], F32R, tag="d_ln2")
    for tc4 in range(4):
        mean, rstd = _ln_stats(nc, phd, xo_tm[:, tc4, :], eps, "d")
        xc = phd.tile([128, C], F32, tag="d_xc")
        nc.vector.tensor_scalar(
            out=xc[:], in0=xo_tm[:, tc4, :], scalar1=mean[:],
            scalar2=rstd[:], op0=OP.subtract, op1=OP.mult)
        for c in range(CK):
            pt = ps_t.tile([128, 128], F32, tag="tp")
            nc.tensor.transpose(pt[:], xc[:, c * 128:(c + 1) * 128], ident_f[:])
            nc.vector.tensor_scalar(
                out=ln2[:, c, tc4 * 128:(tc4 + 1) * 128], in0=pt[:],
                scalar1=g_sb[:, c:c + 1], scalar2=b_sb[:, c:c + 1],
                op0=OP.mult, op1=OP.add)

    # MLP
    h_fm = phd.tile([128, MHK, GT], F32R, tag="d_h")
    for mc in range(MHK):
        ph = ps_m.tile([128, GT], F32, tag="mm")
        for kc in range(CK):
            nc.tensor.matmul(
                ph[:], mw[m, "w1"][:, kc, mc * 128:(mc + 1) * 128],
                ln2[:, kc, :],
                start=(kc == 0), stop=(kc == CK - 1))
        nc.scalar.activation(out=h_fm[:, mc, :], in_=ph[:], func=AF.Gelu,
                             bias=mw[m, "b1"][:, mc:mc + 1], scale=1.0)

    mlp_fm = phd.tile([128, CK, GT], F32, tag="d_mlp")
    for mc in range(CK):
        po = ps_m.tile([128, GT], F32, tag="mm")
        for kc in range(MHK):
            nc.tensor.matmul(
                po[:], mw[m, "w2"][:, kc, mc * 128:(mc + 1) * 128],
                h_fm[:, kc, :],
                start=(kc == 0), stop=(kc == MHK - 1))
        nc.vector.tensor_scalar(
            out=mlp_fm[:, mc, :], in0=po[:], scalar1=mw[m, "b2"][:, mc:mc + 1],
            scalar2=0.0, op0=OP.add, op1=OP.add)

    for tc4 in range(4):
        ot = phd.tile([128, C], F32, tag="d_out")
        for c in range(CK):
            pt = ps_t.tile([128, 128], F32, tag="tp")
            nc.tensor.transpose(pt[:], mlp_fm[:, c, tc4 * 128:(tc4 + 1) * 128],
                                ident_f[:])
            nc.vector.tensor_add(out=ot[:, c * 128:(c + 1) * 128], in0=pt[:],
                                 in1=xo_tm[:, tc4, c * 128:(c + 1) * 128])
        _dma_win_chunk_out(nc, d[outn], ot, g, tc4)


# ---------------------------------------------------------------------------
# host side
# ---------------------------------------------------------------------------

_NC_CACHE = None


def _rel_pos_index(ws):
    coords = np.stack(np.meshgrid(np.arange(ws), np.arange(ws),
                                  indexing="ij")).reshape(2, -1)
    rel = (coords[:, :, None] - coords[:, None, :]).transpose(1, 2, 0).copy()
    rel[:, :, 0] += ws - 1
    rel[:, :, 1] += ws - 1
    rel[:, :, 0] *= 2 * ws - 1
    return rel.sum(-1)


def _prep_shared(inputs):
    """Host-side layout prep of weights (shared across cores)."""
    import ml_dtypes
    bf = ml_dtypes.bfloat16
    sh = {}
    rpi = _rel_pos_index(WS)
    for br in BRANCHES:
        qw = inputs[br + "_qw"]
        kvw = inputs[br + "_kvw"]
        kvb = inputs[br + "_kvb"]
        sh[br + "_qw"] = np.ascontiguousarray(
            qw.reshape(CK, 128, C)).astype(bf)
        sh[br + "_kw"] = np.ascontiguousarray(
            (kvw[:, :C] * SCALE).reshape(CK, 128, C)).astype(bf)
        sh[br + "_vw"] = np.ascontiguousarray(
            kvw[:, C:].reshape(CK, 128, C)).astype(bf)
        sh[br + "_pw"] = np.ascontiguousarray(
            inputs[br + "_pw"].reshape(CK, 128, C)).astype(np.float32)
        sh[br + "_qb"] = np.ascontiguousarray(
            inputs[br + "_qb"].reshape(CK, 128).T).astype(np.float32)
        sh[br + "_kb"] = np.ascontiguousarray(
            (kvb[:C] * SCALE).reshape(CK, 128).T).astype(np.float32)
        sh[br + "_vb"] = kvb[C:].reshape(1, C).astype(bf)
        sh[br + "_pb"] = np.ascontiguousarray(
            inputs[br + "_pb"].reshape(CK, 128).T).astype(np.float32)
        # rel-pos bias, transposed layout [tk, h, tq], duplicated across halves
        bfull = np.asarray(inputs[br + "_rpb"])[rpi]       # [n, m, NH]
        btr = np.transpose(bfull, (1, 2, 0))               # [m(tk), NH, n(tq)]
        sh[br + "_bt"] = np.concatenate([btr, btr], axis=0).astype(np.float32)
    for m in ("m1", "m2"):
        sh[m + "_w1"] = np.ascontiguousarray(
            inputs[m + "_w1"].reshape(CK, 128, MLP_H)).astype(np.float32)
        sh[m + "_b1"] = np.ascontiguousarray(
            inputs[m + "_b1"].reshape(MHK, 128).T).astype(np.float32)
        sh[m + "_w2"] = np.ascontiguousarray(
            inputs[m + "_w2"].reshape(MHK, 128, C)).astype(np.float32)
        sh[m + "_b2"] = np.ascontiguousarray(
            inputs[m + "_b2"].reshape(CK, 128).T).astype(np.float32)
    for ln in ("n11", "n12", "n21", "n22"):
        sh[ln + "_g"] = np.ascontiguousarray(
            inputs[ln + "_g"].reshape(CK, 128).T).astype(np.float32)
        sh[ln + "_b"] = np.ascontiguousarray(
            inputs[ln + "_b"].reshape(CK, 128).T).astype(np.float32)
    sh["ident_bf"] = np.eye(128, dtype=bf)
    sh["ident_f32"] = np.eye(128, dtype=np.float32)
    ho = np.zeros((128, 2, 64), dtype=bf)
    ho[0:64, 0, :] = 1
    ho[64:128, 1, :] = 1
    sh["halfones"] = ho
    sh["onesrow"] = np.ones((1, 128), dtype=bf)
    return sh


def kernel(**inputs):
    global _NC_CACHE
    if _NC_CACHE is None:
        _NC_CACHE = build_nc()
    nc = _NC_CACHE

    sh = _prep_shared(inputs)
    in_maps = []
    for b in range(B):
        im = dict(sh)
        im["x"] = np.ascontiguousarray(inputs["x"][b]).astype(np.float32)
        im["y"] = np.ascontiguousarray(inputs["y"][b]).astype(np.float32)
        for s in ("x", "y"):
            qg = np.asarray(inputs[s + "_q"])[b, 0]        # [NH, N, HD]
            qfm = qg.transpose(0, 2, 1).reshape(C, N)      # [(h d), n]
            im["qg_" + s] = np.ascontiguousarray(
                qfm.reshape(CK, 128, N).transpose(1, 0, 2)).astype(np.float32)
        in_maps.append(im)

    res = run_bass_kernel_spmd(nc, in_maps, core_ids=list(range(B)))
    xo = np.stack([res.results[b]["xo"] for b in range(B)])
    yo = np.stack([res.results[b]["yo"] for b in range(B)])
    return xo.astype(np.float32), yo.astype(np.float32)
